# revision 15
# baseline (speedup 1.0000x reference)
"""Trainium2 Bass kernel for 16-head cross attention, tensor-parallel over 8 cores.

Reference computation (fp32):
    q = (x @ Wq).reshape(n, 16, 64)   # x [2048, 1024], Wq [1024, 1024]
    k = (ctx @ Wk).reshape(m, 16, 64) # ctx [2048, 768]
    v = (ctx @ Wv).reshape(m, 16, 64)
    out[h] = softmax(q[h] @ k[h].T / 8) @ v[h]
    y = out.reshape(n, 1024) @ Wo
Sharding: heads split 2-per-core (columns of Wq/Wk/Wv, rows of Wo). Each core
produces a partial y (transposed); the host sums the 8 partials.

Per-core pipeline (v2 — ACT/PE co-scheduled):
  - Scores are computed transposed (scoresT [m, n]) so the PV contraction (m)
    lands on partitions; softmax denominators come from a ones-column in v;
    no max subtraction (scores ~ N(0,1), fp32 exp is safe).
  - The n axis runs in 4 blocks of 512. Within a block, PV(mt) is issued TWO
    iterations behind scores(mt) so the exp(mt) on the Scalar engine never
    sits in the PE's in-order dependency chain: ACT streams exps
    back-to-back while the PE streams the next scores.
  - kT/qT/v/Wo-proj matmuls are spread one-or-two per iteration into the
    PE slack of the exp cadence, not emitted as lumps.
  - At block end the PV psum is evacuated to SBUF immediately (copy), so the
    2 PV banks recycle into the next block while the normalize (recip ->
    partition-broadcast -> mul) runs on DVE/GpSimd off the critical path.
  - PSUM budget: score ping-pong 2x2 banks + PV 2 + aux(v/proj) 1 +
    kT/qT-emit 1 = 8 banks.
  - Inputs arrive as 9 coarse DMAs on the sync/vector/gpsimd queues
    (scalar queue stays free for exp), ordered so the block-0 needs land
    first.
"""

import os
import sys

for _p in ("/opt/trn_rl_repo", "/root/.axon_site/_ro/trn_rl_repo"):
    if os.path.isdir(_p) and _p not in sys.path:
        sys.path.insert(0, _p)

import numpy as np
import ml_dtypes

import concourse.bass as bass
import concourse.mybir as mybir
import concourse.tile as tile
from concourse import bacc
from concourse.bass_utils import run_bass_kernel_spmd

P = 128
N_TOK = 2048  # n: query rows
M_TOK = 2048  # m: context rows
D = 1024
C = 768
HEADS = 16
DH = 64  # head dim
HPC = 2  # heads per core
SCALE = 8.0  # sqrt(DH)

NB = 512  # n-block width for the attention phase
DK = D // P  # 8 contraction chunks for x projections
CK = C // P  # 6 contraction chunks for ctx projections
MT = M_TOK // P  # 16 context chunks
NBLK = N_TOK // NB  # 4
AT_LEAD = 2  # PV trails scores by this many mt iterations

DTYPE_MODE = os.environ.get("CA_DTYPE", "bf16")


def _dtypes():
    if DTYPE_MODE == "bf16":
        return mybir.dt.bfloat16, ml_dtypes.bfloat16, mybir.dt.bfloat16
    if DTYPE_MODE == "f32r":
        return mybir.dt.float32r, np.float32, mybir.dt.float32r
    return mybir.dt.float32, np.float32, mybir.dt.float32


def _mm_cast(ap, mm_dt):
    return ap.bitcast(mm_dt) if ap.dtype != mm_dt else ap


def build_core_program():
    dt_store, _, dt_mm = _dtypes()
    f32 = mybir.dt.float32

    nc = bacc.Bacc("TRN2", target_bir_lowering=False, debug=False)

    xT = nc.declare_dram_parameter("xT", [D, N_TOK], dt_store, isOutput=False)
    ctxT = nc.declare_dram_parameter("ctxT", [C, M_TOK], dt_store, isOutput=False)
    wq = nc.declare_dram_parameter("wq", [D, P], dt_store, isOutput=False)
    wk = nc.declare_dram_parameter("wk", [C, P], dt_store, isOutput=False)
    wv = nc.declare_dram_parameter("wv", [C, P], dt_store, isOutput=False)
    wo = nc.declare_dram_parameter("wo", [P, D], dt_store, isOutput=False)
    yT = nc.declare_dram_parameter("yT", [D, N_TOK], f32, isOutput=True)

    with tile.TileContext(nc) as tc:
        with (
            tc.tile_pool(name="wts", bufs=1) as wts,
            tc.tile_pool(name="att", bufs=4) as att,
            tc.tile_pool(name="yout", bufs=3) as yout,
            tc.tile_pool(name="small", bufs=2) as small,
            tc.tile_pool(name="ps_sc", bufs=2, space="PSUM") as ps_sc,  # 2x2 banks
            tc.tile_pool(name="ps_pv", bufs=2, space="PSUM") as ps_pv,  # 2x1
            tc.tile_pool(name="ps_aux", bufs=1, space="PSUM") as ps_aux,  # 1
            tc.tile_pool(name="ps_emit", bufs=1, space="PSUM") as ps_emit,  # 1
        ):
            # ---- input DMA: 9 coarse transfers; weights arrive host
            # pre-shuffled as [P, o, e]; big tensors as [P, o, m] column
            # pieces so block-0 dependencies land first. Per-queue DMA
            # rings serialize transfers, so the pieces are spread across
            # sync/scalar/gpsimd by need-time; the scalar queue only
            # carries the two earliest-needed transfers. ----
            wk_sb = wts.tile([P, CK, P], dt_store)
            nc.sync.dma_start(wk_sb[:], wk.ap().rearrange("(p o) e -> p o e", o=CK))
            wq_sb = wts.tile([P, DK, P], dt_store)
            nc.scalar.dma_start(wq_sb[:], wq.ap().rearrange("(p o) e -> p o e", o=DK))

            ctxT_sb = wts.tile([P, CK, M_TOK], dt_store)
            ctx_src = ctxT.ap().rearrange("(o p) m -> p o m", p=P)
            nc.sync.dma_start(ctxT_sb[:, :, :NB], ctx_src[:, :, :NB])
            xT_sb = wts.tile([P, DK, N_TOK], dt_store)
            x_src = xT.ap().rearrange("(o p) m -> p o m", p=P)
            nc.scalar.dma_start(xT_sb[:, :, :NB], x_src[:, :, :NB])

            # ACT exp-table preload: tiny dummy exp, after the scalar
            # queue's DMA issues so they hit the ring first; the ~1.3us
            # ACT_TABLE_LOAD hides under the input DMA
            warm = small.tile([1, 8], f32, tag="warm", bufs=1)
            nc.vector.memset(warm[:], 0.0)
            nc.scalar.activation(warm[:], warm[:], mybir.ActivationFunctionType.Exp)

            wv_sb = wts.tile([P, CK, P], dt_store)
            nc.gpsimd.dma_start(wv_sb[:], wv.ap().rearrange("(p o) e -> p o e", o=CK))
            nc.sync.dma_start(
                ctxT_sb[:, :, NB : 2 * NB], ctx_src[:, :, NB : 2 * NB]
            )
            nc.gpsimd.dma_start(
                ctxT_sb[:, :, 2 * NB : 3 * NB], ctx_src[:, :, 2 * NB : 3 * NB]
            )
            nc.sync.dma_start(ctxT_sb[:, :, 3 * NB :], ctx_src[:, :, 3 * NB :])
            nc.gpsimd.dma_start(xT_sb[:, :, NB:], x_src[:, :, NB:])
            wo_sb = wts.tile([P, D], dt_store)
            nc.gpsimd.dma_start(wo_sb[:], wo.ap())

            # ---- persistent intermediates ----
            kT_sb = wts.tile([P, M_TOK], dt_store)  # [dh(2 heads), m]
            qT_sb = wts.tile([P, N_TOK], dt_store)  # [dq(2 heads), n]
            # v_aug layout [m, mt, 128]: col 0 = ones (softmax sums land on
            # PSUM partition 0), cols 64..127 = v values (normalize reads a
            # window that doesn't straddle the 64-partition boundary)
            VW = 128
            vA_sb = wts.tile([P, MT, VW], dt_store)
            vB_sb = wts.tile([P, MT, VW], dt_store)
            oT_sb = wts.tile([P, N_TOK], dt_store)  # attn out^T, both heads

            def _memset(ap, val):
                if ap.dtype == mybir.dt.float32r:
                    ap = ap.bitcast(f32)
                nc.vector.memset(ap, val)

            _memset(vA_sb[:], 0.0)
            _memset(vB_sb[:], 0.0)
            _memset(vA_sb[:, :, 0:1], 1.0)
            _memset(vB_sb[:, :, 0:1], 1.0)


            def mm(out, lhsT, rhs, start, stop):
                nc.tensor.matmul(
                    out, _mm_cast(lhsT, dt_mm), _mm_cast(rhs, dt_mm),
                    start=start, stop=stop,
                )

            # ---- staged kT/qT emission: a few matmuls per attention
            # iteration into the 1-bank emit psum, copy on the last ----
            emit_ps = {}

            def kq_step(kind, j, lo, hi):
                nch = CK if kind == "k" else DK
                w_sb = wk_sb if kind == "k" else wq_sb
                src = ctxT_sb if kind == "k" else xT_sb
                dst = kT_sb if kind == "k" else qT_sb
                key = (kind, j)
                if lo == 0:
                    emit_ps[key] = ps_emit.tile(
                        [P, NB], f32, tag="emit", name=f"ps_{kind}{j}"
                    )
                ps = emit_ps[key]
                for c in range(lo, hi):
                    mm(ps, w_sb[:, c, :], src[:, c, j * NB : (j + 1) * NB],
                       start=(c == 0), stop=(c == nch - 1))
                if hi == nch:
                    nc.vector.tensor_copy(dst[:, j * NB : (j + 1) * NB], ps)

            def emit_v(mt):
                ps = ps_aux.tile([P, P], f32, tag="aux", name="ps_v")
                for ck in range(CK):
                    mm(ps, ctxT_sb[:, ck, mt * P : (mt + 1) * P], wv_sb[:, ck, :],
                       start=(ck == 0), stop=(ck == CK - 1))
                nc.vector.tensor_copy(vA_sb[:, mt, 64 : 64 + DH], ps[:, :DH])
                nc.vector.tensor_copy(vB_sb[:, mt, 64 : 64 + DH], ps[:, DH:])

            def emit_proj(s, nsl_prev):
                ps = ps_aux.tile([P, NB], f32, tag="aux", name="ps_proj")
                mm(ps, wo_sb[:, s * P : (s + 1) * P], oT_sb[:, nsl_prev],
                   start=True, stop=True)
                ys = yout.tile([P, NB], f32, tag="yout", name="ys")
                nc.vector.tensor_copy(ys[:], ps[:])
                nc.sync.dma_start(yT.ap()[s * P : (s + 1) * P, nsl_prev], ys[:])

            # per-(nb, mt) extra PE work, sized to the ACT exp cadence
            extras = {}

            def add_extra(nb, mt, fn):
                extras.setdefault((nb, mt), []).append(fn)

            # block 0 also produces kT(1..3) (needed at mt 4/8/12) and qT(1)
            for j in (1, 2, 3):
                base = (j - 1) * 3
                for i, (lo, hi) in enumerate(((0, 2), (2, 4), (4, 6))):
                    add_extra(0, base + i,
                              lambda j=j, lo=lo, hi=hi: kq_step("k", j, lo, hi))
            for i, (lo, hi) in enumerate(((0, 2), (2, 4), (4, 6), (6, 8))):
                add_extra(0, 9 + i, lambda lo=lo, hi=hi: kq_step("q", 1, lo, hi))
            # blocks 1-2 produce qT(nb+1)
            for nbb in (1, 2):
                for i, (lo, hi) in enumerate(((0, 2), (2, 4), (4, 6), (6, 8))):
                    add_extra(nbb, 8 + i,
                              lambda j=nbb + 1, lo=lo, hi=hi: kq_step("q", j, lo, hi))
            # blocks 1-3 run the previous block's Wo projection at mt 6..13
            for nbb in (1, 2, 3):
                for s in range(8):
                    add_extra(
                        nbb, 6 + s,
                        lambda s=s, nbb=nbb: emit_proj(
                            s, slice((nbb - 1) * NB, nbb * NB)
                        ),
                    )

            # ---- prologue: kT(0), qT(0) through the score-psum rotation ----
            with nc.named_scope("prologue"):
                pk = ps_sc.tile([P, 2, NB], f32, tag="sc", name="pk")[:, 0, :]
                for ck in range(CK):
                    mm(pk, wk_sb[:, ck, :], ctxT_sb[:, ck, :NB],
                       start=(ck == 0), stop=(ck == CK - 1))
                nc.vector.tensor_copy(kT_sb[:, :NB], pk)
                pq = ps_sc.tile([P, 2, NB], f32, tag="sc", name="pq")[:, 0, :]
                for dk in range(DK):
                    mm(pq, wq_sb[:, dk, :], xT_sb[:, dk, :NB],
                       start=(dk == 0), stop=(dk == DK - 1))
                nc.vector.tensor_copy(qT_sb[:, :NB], pq)

            # ---- attention blocks ----
            def emit_pv(pvA, pvB, at, j):
                st, sp = (j == 0), (j == MT - 1)
                mm(pvA[:], vA_sb[:, j, :], at[:, 0, :], start=st, stop=sp)
                mm(pvB[:], vB_sb[:, j, :], at[:, 1, :], start=st, stop=sp)

            for nb in range(NBLK):
                nsl = slice(nb * NB, (nb + 1) * NB)
                with nc.named_scope(f"att{nb}"):
                    pvA = ps_pv.tile([P, NB], f32, tag="pv", name="pvA")
                    pvB = ps_pv.tile([P, NB], f32, tag="pv", name="pvB")
                    at_ring = {}
                    for mt in range(MT):
                        msl = slice(mt * P, (mt + 1) * P)
                        sc = ps_sc.tile([P, 2, NB], f32, tag="sc", name="sc")
                        mm(sc[:, 0, :], kT_sb[0:DH, msl], qT_sb[0:DH, nsl],
                           start=True, stop=True)
                        mm(sc[:, 1, :], kT_sb[DH:P, msl], qT_sb[DH:P, nsl],
                           start=True, stop=True)
                        at = att.tile([P, 2, NB], dt_store, tag="at", name="at")
                        nc.scalar.activation(
                            at[:], sc[:], mybir.ActivationFunctionType.Exp
                        )
                        at_ring[mt] = at
                        if nb == 0:
                            emit_v(mt)
                        for fn in extras.get((nb, mt), ()):
                            fn()
                        j = mt - AT_LEAD
                        if j >= 0:
                            emit_pv(pvA, pvB, at_ring.pop(j), j)
                    for j in range(MT - AT_LEAD, MT):
                        emit_pv(pvA, pvB, at_ring.pop(j), j)
                    if nb < NBLK - 1:
                        # evacuate PV psum (frees the banks for the next
                        # block), then normalize from SBUF off-path
                        evA = small.tile([P, NB], f32, tag="evac", name="evA",
                                         bufs=2)
                        nc.vector.tensor_copy(evA[:], pvA[:])
                        evB = small.tile([P, NB], f32, tag="evac", name="evB",
                                         bufs=2)
                        nc.vector.tensor_copy(evB[:], pvB[:])
                        for h, ev in ((0, evA), (1, evB)):
                            rcf = small.tile([1, NB], f32, tag="recip",
                                             name="rcf")
                            nc.vector.reciprocal_approx_fast(rcf[:], ev[0:1, :])
                            bcs = small.tile([P, NB], f32, tag="bcast",
                                             name="bcs")
                            nc.gpsimd.partition_broadcast(bcs[:], rcf[:])
                            nc.vector.tensor_mul(
                                oT_sb[h * DH : (h + 1) * DH, nsl],
                                ev[64 : 64 + DH, :], bcs[64:P, :],
                            )

            # ---- tail: last block normalizes straight from PSUM (the mul
            # mixes PSUM + SBUF inputs, so a 64-row broadcast suffices),
            # then the projection streams through the score-psum ping-pong
            # with DVE/Scalar copies ----
            with nc.named_scope("tail"):
                nsl3 = slice((NBLK - 1) * NB, NBLK * NB)
                for h, pv in ((0, pvA), (1, pvB)):
                    rcf = small.tile([1, NB], f32, tag="recip", name="rcf")
                    nc.vector.reciprocal_approx_fast(rcf[:], pv[0:1, :])
                    bcs = small.tile([DH, NB], f32, tag="bcast_t", name="bcs",
                                     bufs=2)
                    nc.gpsimd.partition_broadcast(bcs[:], rcf[:])
                    nc.vector.tensor_mul(
                        oT_sb[h * DH : (h + 1) * DH, nsl3],
                        pv[64 : 64 + DH, :], bcs[:],
                    )
                for s in range(8):
                    ps = ps_sc.tile([P, 2, NB], f32, tag="sc", name="ps_ty")[:, 0, :]
                    mm(ps, wo_sb[:, s * P : (s + 1) * P], oT_sb[:, nsl3],
                       start=True, stop=True)
                    ys = yout.tile([P, NB], f32, tag="yout", name="ys")
                    if s % 2 == 0:
                        nc.vector.tensor_copy(ys[:], ps[:])
                    else:
                        nc.scalar.copy(ys[:], ps[:])
                    nc.sync.dma_start(yT.ap()[s * P : (s + 1) * P, nsl3], ys[:])

    nc.compile()
    return nc


_NC_CACHE = {}


def _get_nc():
    key = DTYPE_MODE
    if key not in _NC_CACHE:
        _NC_CACHE[key] = build_core_program()
    return _NC_CACHE[key]


def _shuffle_w(w):
    # [o*P + p, e] -> [p*o_n + o, e] so each SBUF partition's rows are
    # contiguous in DRAM (single contiguous DMA into a [P, o, e] tile)
    o_n = w.shape[0] // P
    return np.ascontiguousarray(
        w.reshape(o_n, P, w.shape[1]).transpose(1, 0, 2).reshape(w.shape)
    )


def _prep_in_maps(x, ctx, Wq, Wk, Wv, Wo):
    _, np_dt, _ = _dtypes()
    xT = np.ascontiguousarray(x.T).astype(np_dt)
    ctxT = np.ascontiguousarray(ctx.T).astype(np_dt)
    Wq_s = (Wq / SCALE).astype(np.float32)
    in_maps = []
    for cc in range(8):
        csl = slice(cc * P, (cc + 1) * P)
        in_maps.append(
            {
                "xT": xT,
                "ctxT": ctxT,
                "wq": _shuffle_w(np.ascontiguousarray(Wq_s[:, csl])).astype(np_dt),
                "wk": _shuffle_w(np.ascontiguousarray(Wk[:, csl])).astype(np_dt),
                "wv": _shuffle_w(np.ascontiguousarray(Wv[:, csl])).astype(np_dt),
                "wo": np.ascontiguousarray(Wo[csl, :]).astype(np_dt),
            }
        )
    return in_maps


def run(x, ctx, Wq, Wk, Wv, Wo, trace=False):
    nc = _get_nc()
    in_maps = _prep_in_maps(x, ctx, Wq, Wk, Wv, Wo)
    res = run_bass_kernel_spmd(nc, in_maps, core_ids=list(range(8)), trace=trace)
    acc = np.zeros((D, N_TOK), np.float32)
    for r in res.results:
        acc += r["yT"]
    return np.ascontiguousarray(acc.T), res


def kernel(x, ctx, Wq, Wk, Wv, Wo):
    x = np.asarray(x, dtype=np.float32)
    ctx = np.asarray(ctx, dtype=np.float32)
    Wq = np.asarray(Wq, dtype=np.float32)
    Wk = np.asarray(Wk, dtype=np.float32)
    Wv = np.asarray(Wv, dtype=np.float32)
    Wo = np.asarray(Wo, dtype=np.float32)
    y, _ = run(x, ctx, Wq, Wk, Wv, Wo, trace=False)
    return y
